# revision 2
# baseline (speedup 1.0000x reference)
"""GCN layer (GCNConv + BatchNorm1d + ReLU + residual) on 8 Trainium2 cores, v3.

Strategy (v3):
  - Nodes sharded 8 ways (6250/core); edges partitioned by destination core,
    bucketed by destination block (49 blocks of 128 nodes).
  - All normalization is folded into per-edge weights computed on host:
    h[dst] = sum_e norm[e] * x[src_e] (+ dinv^2[dst] * x[dst])  then @ W^T.
  - Gather: bf16, pair-addressed (idx = src>>1 fits int16; each descriptor
    fetches the 512B pair-row [x[2k] | x[2k+1]]), issued as 8192-idx
    dma_gather instructions (SWDGE); per-edge parity resolved by splitting
    the selection matrix into even/odd columns (weights we/wo, one zero).
  - Per tile of 128 edges: s_cat[e,(j,d)] = (iota==cr[e]) * w_j[e] built in a
    single DVE op; two bf16 matmuls accumulate aggT[f,d] in PSUM.
  - Self-loop via PE transpose-accumulate of xres*dinv2 (bf16).
  - Per dst block: fin = aggT^T @ W^T (fp32); BN batch stats accumulate via
    ones-vector matmuls; one 8-core AllReduce of [2,128]; finalize
    out = x + relu(h*s + t) in 4 chunks pipelined across DVE/Scalar.
  - Host side does only index/sharding prep + dtype cast of x to bf16.
"""

import sys

sys.path.insert(0, "/opt/trn_rl_repo")

import numpy as np
import ml_dtypes

import concourse.bacc as bacc
import concourse.mybir as mybir
import concourse.tile as tile
from concourse.bass_utils import run_bass_kernel_spmd
from concourse.masks import make_identity

P = 128
D = 128
F32 = mybir.dt.float32
BF16 = mybir.dt.bfloat16
I16 = mybir.dt.int16
BN_EPS = 1e-5
CORES = 8
SBW = 5  # dst blocks per psum group (5 agg + fin + sh + sh2 = 8 banks)
GCH = 8192  # main gather chunk (idxs per dma_gather)
BF = ml_dtypes.bfloat16


# ---------------------------------------------------------------- host prep
def _build_plan(x, edge_index, n_nodes):
    N = n_nodes
    npc = N // CORES
    nblk = (npc + P - 1) // P
    npad_local = nblk * P
    NPAD = ((N + P - 1) // P) * P
    NH = NPAD // 2
    assert NH - 1 <= 32767

    src = np.asarray(edge_index[0]).astype(np.int64).astype(np.int32)
    dst = np.asarray(edge_index[1]).astype(np.int64).astype(np.int32)
    deg = (np.bincount(dst, minlength=N) + 1).astype(np.float32)
    dinv = 1.0 / np.sqrt(deg)

    core_of = dst // npc
    dloc = dst - core_of * npc
    db_l = dloc // P

    order = np.lexsort((db_l, core_of))
    src_s = src[order]
    dloc_s = dloc[order]
    core_s = core_of[order]
    db_s = db_l[order]
    norm_s = (dinv[src_s] * dinv[dst[order]]).astype(np.float32)

    cnt = np.zeros((CORES, nblk), np.int64)
    np.add.at(cnt, (core_s, db_s), 1)
    T = ((cnt.max(axis=0) + P - 1) // P).astype(np.int64)  # [nblk]
    tiles_total = int(T.sum())
    tot_e = tiles_total * P
    s_tot = tot_e // 16

    offs = np.zeros((CORES, nblk), np.int64)
    run = 0
    for c in range(CORES):
        for db in range(nblk):
            offs[c, db] = run
            run += cnt[c, db]
    assert run == src.shape[0]

    slot_pos = np.zeros(nblk, np.int64)
    pos = 0
    for db in range(nblk):
        slot_pos[db] = pos
        pos += int(T[db]) * P
    assert pos == tot_e

    idx_streams = np.zeros((CORES, tot_e), np.int16)
    cr_streams = np.full((CORES, tot_e), -1.0, np.float32)
    we_streams = np.zeros((CORES, tot_e), np.float32)
    wo_streams = np.zeros((CORES, tot_e), np.float32)
    for c in range(CORES):
        for db in range(nblk):
            k = int(cnt[c, db])
            if k == 0:
                continue
            o = int(offs[c, db])
            p0 = int(slot_pos[db])
            ss = src_s[o : o + k]
            idx_streams[c, p0 : p0 + k] = (ss >> 1).astype(np.int16)
            cr_streams[c, p0 : p0 + k] = (dloc_s[o : o + k] - db * P).astype(
                np.float32)
            nn = norm_s[o : o + k]
            even = (ss & 1) == 0
            we_streams[c, p0 : p0 + k] = np.where(even, nn, 0.0)
            wo_streams[c, p0 : p0 + k] = np.where(even, 0.0, nn)

    idx16 = np.zeros((CORES, P, s_tot), np.int16)
    cr_pk = np.zeros((CORES, P, tiles_total), np.float32)
    w2_pk = np.zeros((CORES, P, 2 * tiles_total), np.float32)
    for c in range(CORES):
        idx16[c] = np.tile(idx_streams[c].reshape(-1, 16).T, (8, 1))
        cr_pk[c] = cr_streams[c].reshape(-1, P).T
        w2_pk[c, :, 0::2] = we_streams[c].reshape(-1, P).T
        w2_pk[c, :, 1::2] = wo_streams[c].reshape(-1, P).T

    dinv2_own = np.zeros((CORES, P, nblk), np.float32)
    xres = np.zeros((CORES, npad_local, D), np.float32)
    for c in range(CORES):
        dd = np.ones(npad_local, np.float32)
        dd[:npc] = deg[c * npc : (c + 1) * npc]
        dinv2_own[c] = (1.0 / dd).reshape(nblk, P).T
        xres[c, :npc] = x[c * npc : (c + 1) * npc]

    x_pad = np.zeros((NPAD, D), np.float32)
    x_pad[:N] = x
    xb2 = x_pad.reshape(NH, 2 * D).astype(BF)

    # gather chunk schedule: 1024-idx chunks (SWDGE ring-friendly)
    chunks = []
    rem = tot_e
    while rem > 0:
        c = min(1024, rem)
        chunks.append(c)
        rem -= c
    assert sum(chunks) == tot_e

    return dict(
        N=N, npc=npc, nblk=nblk, npad_local=npad_local, NPAD=NPAD, NH=NH,
        T=T, tiles_total=tiles_total, s_tot=s_tot, tot_e=tot_e,
        idx16=idx16, cr_pk=cr_pk, w2_pk=w2_pk, dinv2_own=dinv2_own,
        xres=xres, xb2=xb2, chunks=chunks,
    )


# ------------------------------------------------------------- device build
def _build_program(plan):
    N = plan["N"]
    nblk = plan["nblk"]
    NH = plan["NH"]
    npc, npad_local = plan["npc"], plan["npad_local"]
    T = plan["T"]
    tiles_total, s_tot = plan["tiles_total"], plan["s_tot"]
    chunks = plan["chunks"]

    nc = bacc.Bacc("TRN2", target_bir_lowering=False, debug=False,
                   num_devices=CORES)

    xb2_d = nc.declare_dram_parameter("xb2", [NH, 2 * D], BF16, isOutput=False)
    xres_d = nc.declare_dram_parameter("xres", [npad_local, D], F32,
                                       isOutput=False)
    dinv2_d = nc.declare_dram_parameter("dinv2own", [P, nblk], F32,
                                        isOutput=False)
    W_d = nc.declare_dram_parameter("W", [D, D], F32, isOutput=False)
    gamma_d = nc.declare_dram_parameter("gamma", [1, D], F32, isOutput=False)
    beta_d = nc.declare_dram_parameter("beta", [1, D], F32, isOutput=False)
    idx_d = nc.declare_dram_parameter("idx16", [P, s_tot], I16, isOutput=False)
    cr_d = nc.declare_dram_parameter("crpk", [P, tiles_total], F32,
                                     isOutput=False)
    w2_d = nc.declare_dram_parameter("w2pk", [P, 2 * tiles_total], F32,
                                     isOutput=False)
    out_d = nc.declare_dram_parameter("out", [npc, D], F32, isOutput=True)
    import os as _os
    _DBG = _os.environ.get("KDBG", "") == "1"
    if _DBG:
        dbg_h = nc.declare_dram_parameter("dbg_h", [P, nblk * P], F32,
                                          isOutput=True)


    cc_in = nc.dram_tensor("cc_in", [2, D], F32)
    cc_out = nc.dram_tensor("cc_out", [2, D], F32, addr_space="Shared")

    with tile.TileContext(nc) as tc:
        with tc.tile_pool(name="const", bufs=1) as cpool, \
             tc.tile_pool(name="work", bufs=3) as wpool, \
             tc.tile_pool(name="work2", bufs=2) as w2pool, \
             tc.tile_pool(name="gath", bufs=2) as gpool, \
             tc.tile_pool(name="psum", bufs=1, space="PSUM") as ppool:

            # ---- constants
            iota_i = cpool.tile([P, P], mybir.dt.int32)
            nc.gpsimd.iota(iota_i[:], pattern=[[1, P]], base=0,
                           channel_multiplier=0)
            iota_bf = cpool.tile([P, P], BF16)
            nc.vector.tensor_copy(iota_bf[:], iota_i[:])

            ident = cpool.tile([P, P], F32)
            make_identity(nc, ident[:])
            ident_bf = cpool.tile([P, P], BF16)
            nc.vector.tensor_copy(ident_bf[:], ident[:])

            w_sb = cpool.tile([D, D], F32)
            nc.sync.dma_start(out=w_sb[:], in_=W_d[:, :])
            wt_ps = ppool.tile([D, D], F32, tag="fin")
            nc.tensor.transpose(out=wt_ps[:], in_=w_sb[:], identity=ident[:])
            w_t = cpool.tile([D, D], F32)
            nc.vector.tensor_copy(w_t[:], wt_ps[:])

            dinv2_sb = cpool.tile([P, nblk], F32)
            nc.sync.dma_start(out=dinv2_sb[:], in_=dinv2_d[:, :])

            gamma_sb = cpool.tile([1, D], F32)
            nc.sync.dma_start(out=gamma_sb[:], in_=gamma_d[:, :])
            beta_sb = cpool.tile([1, D], F32)
            nc.sync.dma_start(out=beta_sb[:], in_=beta_d[:, :])

            ones_col = cpool.tile([P, 1], F32)
            nc.vector.memset(ones_col[:], 1.0)
            ones_row = cpool.tile([1, P], F32)
            nc.vector.memset(ones_row[:], 1.0)

            # stream loads: idx split so chunk 0 can start immediately
            idx_sb = cpool.tile([P, s_tot], I16)
            c0_cols = chunks[0] // 16
            nc.sync.dma_start(out=idx_sb[:, :c0_cols], in_=idx_d[:, :c0_cols])
            nc.sync.dma_start(out=idx_sb[:, c0_cols:], in_=idx_d[:, c0_cols:])
            cr_sb = cpool.tile([P, tiles_total], F32)
            nc.sync.dma_start(out=cr_sb[:], in_=cr_d[:, :])
            w2_sb = cpool.tile([P, 2 * tiles_total], F32)
            nc.sync.dma_start(out=w2_sb[:], in_=w2_d[:, :])

            xres_sb = cpool.tile([P, nblk * P], F32)
            nc.sync.dma_start(
                out=xres_sb[:].rearrange("p (a k) -> p a k", k=P),
                in_=xres_d[:, :].rearrange("(a p) k -> p a k", p=P))

            h_buf = cpool.tile([P, nblk * P], F32)

            # ---- gather stream (emitted in chunk order on gpsimd queue)
            gts = []
            o_idx = 0
            for ci, chlen in enumerate(chunks):
                gt = gpool.tile([P, chlen // P, 2 * P], BF16, tag="gt",
                                name=f"gt_{ci}")
                nc.gpsimd.dma_gather(
                    out_ap=gt[:],
                    in_ap=xb2_d[:, :],
                    idxs_ap=idx_sb[:, o_idx // 16 : (o_idx + chlen) // 16],
                    num_idxs=chlen, num_idxs_reg=chlen, elem_size=2 * P,
                    single_packet=(chlen <= 1024))
                gts.append((gt, o_idx // P, chlen // P))  # tile-start, ntiles
                o_idx += chlen

            def gt_slice(t):
                for gt, t0, nt in gts:
                    if t0 <= t < t0 + nt:
                        return gt, t - t0
                raise AssertionError(t)

            # ---- main loop: per dst block, accumulate aggT in psum
            sh_ps = ppool.tile([1, D], F32, tag="sh", name="sh_ps")
            sh2_ps = ppool.tile([1, D], F32, tag="sh2", name="sh2_ps")

            loc = 0
            for db in range(nblk):
                j = db % SBW
                psum = ppool.tile([P, P], F32, tag=f"agg{j}",
                                  name=f"agg_{db}")
                # self-loop: psum += (xres*dinv2)^T
                xo = wpool.tile([P, P], F32, tag="xo", name=f"xo_{db}")
                nc.vector.tensor_scalar_mul(
                    xo[:], xres_sb[:, db * P : (db + 1) * P],
                    dinv2_sb[:, db : db + 1])
                nc.tensor.matmul(out=psum[:], lhsT=xo[:], rhs=ident[:],
                                 is_transpose=True, start=True,
                                 stop=(int(T[db]) == 0))
                for t in range(int(T[db])):
                    gt, slot = gt_slice(loc)
                    s_e = wpool.tile([P, P], BF16, tag="s_e",
                                     name=f"se_{loc}")
                    nc.vector.tensor_scalar(
                        out=s_e[:], in0=iota_bf[:],
                        scalar1=cr_sb[:, loc : loc + 1],
                        scalar2=w2_sb[:, 2 * loc : 2 * loc + 1],
                        op0=mybir.AluOpType.is_equal,
                        op1=mybir.AluOpType.mult)
                    s_o = wpool.tile([P, P], BF16, tag="s_o",
                                     name=f"so_{loc}")
                    nc.vector.tensor_scalar(
                        out=s_o[:], in0=iota_bf[:],
                        scalar1=cr_sb[:, loc : loc + 1],
                        scalar2=w2_sb[:, 2 * loc + 1 : 2 * loc + 2],
                        op0=mybir.AluOpType.is_equal,
                        op1=mybir.AluOpType.mult)
                    last = (t == int(T[db]) - 1)
                    nc.tensor.matmul(
                        out=psum[:], lhsT=gt[:, slot, 0:P],
                        rhs=s_e[:], start=False, stop=False)
                    nc.tensor.matmul(
                        out=psum[:], lhsT=gt[:, slot, P : 2 * P],
                        rhs=s_o[:], start=False, stop=last)
                    loc += 1
                # fin = aggT^T @ W^T  -> h block [node, feat]
                aggt = wpool.tile([P, P], F32, tag="aggt", name=f"aggt_{db}")
                nc.vector.tensor_copy(aggt[:], psum[:])
                fin = ppool.tile([P, P], F32, tag="fin", name=f"fin_{db}")
                nc.tensor.matmul(out=fin[:], lhsT=aggt[:], rhs=w_t[:],
                                 start=True, stop=True)
                hb = h_buf[:, db * P : (db + 1) * P]
                nc.vector.tensor_copy(hb, fin[:])
                sq = wpool.tile([P, P], F32, tag="sq", name=f"sq_{db}")
                nc.vector.tensor_tensor(out=sq[:], in0=hb, in1=hb,
                                        op=mybir.AluOpType.mult)
                nc.tensor.matmul(out=sh_ps[:], lhsT=ones_col[:], rhs=hb,
                                 start=(db == 0), stop=(db == nblk - 1))
                nc.tensor.matmul(out=sh2_ps[:], lhsT=ones_col[:], rhs=sq[:],
                                 start=(db == 0), stop=(db == nblk - 1))
            assert loc == tiles_total

            # ---- BN stats all-reduce + normalize constants
            sh_sb = cpool.tile([1, D], F32)
            nc.vector.tensor_copy(sh_sb[:], sh_ps[:])
            sh2_sb = cpool.tile([1, D], F32)
            nc.vector.tensor_copy(sh2_sb[:], sh2_ps[:])
            nc.sync.dma_start(out=cc_in[0:1, :], in_=sh_sb[:])
            nc.sync.dma_start(out=cc_in[1:2, :], in_=sh2_sb[:])
            nc.gpsimd.collective_compute(
                "AllReduce", mybir.AluOpType.add,
                ins=[cc_in[:]], outs=[cc_out[:]],
                replica_groups=[list(range(CORES))])
            gsum = cpool.tile([1, D], F32)
            gsum2 = cpool.tile([1, D], F32)
            nc.sync.dma_start(out=gsum[:], in_=cc_out[0:1, :])
            nc.sync.dma_start(out=gsum2[:], in_=cc_out[1:2, :])

            mean = cpool.tile([1, D], F32)
            nc.vector.tensor_scalar_mul(mean[:], gsum[:], 1.0 / N)
            eh2 = cpool.tile([1, D], F32)
            nc.vector.tensor_scalar_mul(eh2[:], gsum2[:], 1.0 / N)
            msq = cpool.tile([1, D], F32)
            nc.vector.tensor_tensor(out=msq[:], in0=mean[:], in1=mean[:],
                                    op=mybir.AluOpType.mult)
            var = cpool.tile([1, D], F32)
            nc.vector.tensor_tensor(out=var[:], in0=eh2[:], in1=msq[:],
                                    op=mybir.AluOpType.subtract)
            vare = cpool.tile([1, D], F32)
            nc.vector.tensor_scalar_add(vare[:], var[:], BN_EPS)
            sdev = cpool.tile([1, D], F32)
            nc.scalar.activation(sdev[:], vare[:],
                                 mybir.ActivationFunctionType.Sqrt)
            rstd = cpool.tile([1, D], F32)
            nc.vector.reciprocal(rstd[:], sdev[:])

            st_row = cpool.tile([1, 2 * D], F32)
            nc.vector.tensor_tensor(out=st_row[:, :D], in0=rstd[:],
                                    in1=gamma_sb[:], op=mybir.AluOpType.mult)
            ms = cpool.tile([1, D], F32)
            nc.vector.tensor_tensor(out=ms[:], in0=mean[:],
                                    in1=st_row[:, :D],
                                    op=mybir.AluOpType.mult)
            nc.vector.tensor_tensor(out=st_row[:, D:], in0=beta_sb[:],
                                    in1=ms[:], op=mybir.AluOpType.subtract)
            bc_ps = ppool.tile([P, 2 * D], F32, tag="fin")
            nc.tensor.matmul(out=bc_ps[:], lhsT=ones_row[:], rhs=st_row[:],
                             start=True, stop=True)
            s_rep = cpool.tile([P, D], F32)
            nc.vector.tensor_copy(s_rep[:], bc_ps[:, :D])
            t_rep = cpool.tile([P, D], F32)
            nc.vector.tensor_copy(t_rep[:], bc_ps[:, D:])
            if _DBG:
                nc.sync.dma_start(out=dbg_h[:, :], in_=h_buf[:])

            # ---- phase 3: out = xres + relu(h*s + t), 4 chunks, DVE+Scalar
            bounds = [0, 13, 25, 37, nblk]
            for k in range(4):
                b0, b1 = bounds[k], bounds[k + 1]
                nb = b1 - b0
                sl = slice(b0 * P, b1 * P)
                h3 = h_buf[:, sl].rearrange("p (a k) -> p a k", k=P)
                tmp = w2pool.tile([P, nb, P], F32, tag="p3t", name=f"p3t_{k}")
                nc.vector.tensor_tensor(
                    out=tmp[:], in0=h3,
                    in1=s_rep[:].rearrange("p (o k) -> p o k", o=1).to_broadcast(
                        [P, nb, P]),
                    op=mybir.AluOpType.mult)
                nc.vector.tensor_tensor(
                    out=tmp[:], in0=tmp[:],
                    in1=t_rep[:].rearrange("p (o k) -> p o k", o=1).to_broadcast(
                        [P, nb, P]),
                    op=mybir.AluOpType.add)
                ot = w2pool.tile([P, nb * P], F32, tag="p3o", name=f"p3o_{k}")
                nc.scalar.activation(
                    ot[:], tmp[:].rearrange("p a k -> p (a k)"),
                    mybir.ActivationFunctionType.Relu)
                nc.vector.tensor_tensor(out=ot[:], in0=ot[:],
                                        in1=xres_sb[:, sl],
                                        op=mybir.AluOpType.add)
                lo = b0 * P
                hi = min(npc, b1 * P)
                nv = hi - lo
                full = nv // P
                if full > 0:
                    nc.sync.dma_start(
                        out=out_d[lo : lo + full * P, :].rearrange(
                            "(a p) k -> p a k", p=P),
                        in_=ot[:, : full * P].rearrange(
                            "p (a k) -> p a k", k=P))
                rem = nv - full * P
                if rem > 0:
                    nc.sync.dma_start(
                        out=out_d[lo + full * P : hi, :],
                        in_=ot[:rem, full * P : (full + 1) * P])

    nc.compile()
    return nc


# ------------------------------------------------------------------ driver
_CACHE = {}
TRACE = False
RUN_KWARGS = None
LAST_RESULT = None


def kernel(**inputs):
    x = np.asarray(inputs["x"], np.float32)
    edge_index = np.asarray(inputs["edge_index"])
    W = np.asarray(inputs["W"], np.float32)
    gamma = np.asarray(inputs["gamma"], np.float32)
    beta = np.asarray(inputs["beta"], np.float32)
    # inputs["b"] shifts h uniformly and cancels under batch-norm mean
    # subtraction, so it does not affect the output.
    N = x.shape[0]

    plan = _build_plan(x, edge_index, N)
    key = (N, edge_index.shape[1], plan["tiles_total"],
           tuple(plan["T"].ravel().tolist()))
    if key not in _CACHE:
        _CACHE[key] = _build_program(plan)
    nc = _CACHE[key]

    in_maps = []
    for c in range(CORES):
        in_maps.append({
            "xb2": plan["xb2"],
            "xres": plan["xres"][c],
            "dinv2own": plan["dinv2_own"][c],
            "W": W,
            "gamma": gamma.reshape(1, -1),
            "beta": beta.reshape(1, -1),
            "idx16": plan["idx16"][c],
            "crpk": plan["cr_pk"][c],
            "w2pk": plan["w2_pk"][c],
        })

    res = run_bass_kernel_spmd(nc, in_maps, list(range(CORES)),
                               trace=TRACE, **(RUN_KWARGS or {}))
    global LAST_RESULT
    LAST_RESULT = res
    out = np.concatenate([res.results[c]["out"] for c in range(CORES)],
                         axis=0)
    return out.astype(np.float32)
